# revision 11
# baseline (speedup 1.0000x reference)
"""Trainium2 Bass kernel for the attention+Mamba hybrid block.

Sharding over 8 NeuronCores: 2 batch groups x 4 cores (replica groups
[[0..3],[4..7]]). Within a group, proj/qkv are L-sharded, scores/softmax/ctx
head-sharded (AllToAll exchanges), LN L-sharded, the Mamba block
channel-sharded (d_inner/4 per core) with the selective scan run as hardware
tensor_tensor_scan over (channel x state) lanes.

Core c: batch b=c//4, quarter q=c%4
  - owns L rows   [512q, 512q+512)
  - owns heads    {2q, 2q+1}
  - owns channels [1024q, 1024q+1024)
Outputs per core: out_part = out^T partial [2048, 2048] fp32 over its channel
slice, ao_out = attn^T [2048, 512] fp32 for its L rows. Host: per batch,
out^T = sum(out_part) ; out^T[:, own cols] += ao_out ; transpose.
"""
import numpy as np
from contextlib import ExitStack

import concourse.bass as bass
import concourse.mybir as mybir
import concourse.tile as tile
import concourse.bacc as bacc
from concourse import bass_utils

AF = mybir.ActivationFunctionType
ALU = mybir.AluOpType
FP32 = mybir.dt.float32
BF16 = mybir.dt.bfloat16

DM = 2048
L = 2048
BATCH = 2
H = 8
HD = 256
DI = 4096
DS = 16
DC = 4
DTR = 128
NCORES = 8
GROUPS = [[0, 1, 2, 3], [4, 5, 6, 7]]
LQ = 512
CH = 1024
TH = 1024

S_PROJB = 0
S_QKB = 16
S_AOB = 48
S_BZ = 64
S_CONVW = 72
S_CONVB = 104
S_DTB = 112
S_D = 120
S_EPS = 128
NSMALL = 132


def _col_blocks(wT, n_ct, n_pt, pw=128):
    c, p = wT.shape
    assert c == n_ct * 128 and p == n_pt * pw
    return np.ascontiguousarray(
        wT.reshape(n_ct, 128, n_pt, pw).transpose(2, 1, 0, 3))


def _row_tiles(a, n_rt):
    r, f = a.shape
    assert r == n_rt * 128
    return np.ascontiguousarray(a.reshape(n_rt, 128, f))


def build_program(debug=False):
    nc = bacc.Bacc("TRN2", target_bir_lowering=False, debug=False,
                   num_devices=NCORES)

    def din(name, shape, dt=BF16):
        return nc.dram_tensor(name, shape, dt, kind="ExternalInput").ap()

    xT = din("xT", [16, 128, LQ])
    wproj = din("wproj", [16, 128, 16, 128])
    wqk = din("wqk", [32, 128, 16, 128])
    wv = din("wv", [16, 128, DM])
    wao = din("wao", [16, 128, 16, 128])
    win = din("win", [16, 128, 16, 128])
    wxp = din("wxp", [8, 128, 256])
    wdt = din("wdt", [128, 8, 128])
    wout = din("wout", [16, 128, 8, 128])
    vbias = din("vbias", [1, DM])
    identin = din("identin", [128, 128])
    smalls = din("smalls", [128, NSMALL], FP32)

    out_part = nc.dram_tensor("out_part", [16, 128, DM], FP32,
                              kind="ExternalOutput").ap()
    ao_out = nc.dram_tensor("ao_out", [16, 128, LQ], FP32,
                            kind="ExternalOutput").ap()

    def dint(name, shape, dt=BF16, shared=False):
        return nc.dram_tensor(name, shape, dt, kind="Internal",
                              addr_space="Shared" if shared else "Local").ap()

    a2a_qk_i = dint("a2a_qk_i", [8, 2, 128, 1024])
    a2a_qk_o = dint("a2a_qk_o", [8, 2, 128, 1024])
    a2a_v_i = dint("a2a_v_i", [8, 128, 1024])
    a2a_v_o = dint("a2a_v_o", [8, 128, 1024])
    a2a_ctx_i = dint("a2a_ctx_i", [8, 128, 1024])
    a2a_ctx_o = dint("a2a_ctx_o", [8, 128, 1024])
    ag_xn_i = dint("ag_xn_i", [2, 8, 128, LQ])
    ag_xn_o = dint("ag_xn_o", [2, 4, 8, 128, LQ])
    ar_xd_i = dint("ar_xd_i", [2, 128, L], FP32)
    ar_xd_o = dint("ar_xd_o", [2, 128, L], FP32)
    bc_bf = dint("bc_bf", [32, L])
    delta_d = dint("delta_d", [8, 128, L], FP32)
    du_d = dint("du_d", [8, 128, L])
    uc_d = dint("uc_d", [8, 128, L])
    sz_d = dint("sz_d", [8, 128, L])
    y_d = dint("y_d", [8, 128, L])

    dbg = {}
    if debug:
        for nm, shp in [("hT", [16, 128, LQ]), ("qkT", [32, 128, LQ]),
                        ("ao", [16, 128, LQ]), ("xn", [16, 128, LQ]),
                        ("uz", [16, 128, L]), ("uc", [8, 128, L]),
                        ("delta", [8, 128, L]), ("xdbl", [2, 128, L]),
                        ("y", [8, 128, L]), ("ctxa", [16, 128, LQ])]:
            dbg[nm] = nc.dram_tensor("dbg_" + nm, shp, FP32,
                                     kind="ExternalOutput").ap()

    with tile.TileContext(nc) as tc, ExitStack() as top:
        persist = top.enter_context(tc.tile_pool(name="persist", bufs=1))
        sm = persist.tile([128, NSMALL], FP32)
        nc.gpsimd.dma_start(sm[:], smalls)
        ones_t = persist.tile([128, 1], BF16)
        nc.gpsimd.memset(ones_t[:], 1.0)
        ones_row = persist.tile([1, 128], BF16)
        nc.gpsimd.memset(ones_row[:], 1.0)
        ident = persist.tile([128, 128], BF16)
        nc.gpsimd.dma_start(ident[:], identin)
        vb_t = persist.tile([1, DM], BF16)
        nc.gpsimd.dma_start(vb_t[:], vbias)

        # ---------------- Phase A: proj, qkv, v ----------------
        with tc.tile_pool(name="phA", bufs=1) as pA, \
             tc.tile_pool(name="wstream", bufs=3) as wsp, \
             tc.tile_pool(name="psH", bufs=4, space="PSUM") as psH, \
             tc.tile_pool(name="psV", bufs=1, space="PSUM") as psV:
            xT_t = pA.tile([128, 16, LQ], BF16)
            nc.sync.dma_start(xT_t[:], xT.rearrange("ct p f -> p ct f"))
            hT = pA.tile([128, 16, LQ], BF16)
            for pi in range(16):
                wb = wsp.tile([128, 16 * 128], BF16, tag="wblk")
                nc.sync.dma_start(wb[:], wproj[pi].rearrange("c ct p -> c (ct p)"))
                ps = psH.tile([128, LQ], FP32)
                for ci in range(16):
                    nc.tensor.matmul(ps[:], wb[:, ci * 128:(ci + 1) * 128],
                                     xT_t[:, ci], start=(ci == 0), stop=(ci == 15))
                nc.scalar.activation(hT[:, pi], ps[:], AF.Identity,
                                     bias=sm[:, S_PROJB + pi:S_PROJB + pi + 1])
            if debug:
                hTf = pA.tile([128, 16, LQ], FP32, tag="hTf")
                for pi in range(16):
                    nc.vector.tensor_copy(hTf[:, pi], hT[:, pi])
                nc.sync.dma_start(dbg["hT"].rearrange("ct p f -> p ct f"), hTf[:])

            qkT = pA.tile([128, 32, LQ], BF16)
            for pi in range(32):
                wb = wsp.tile([128, 16 * 128], BF16, tag="wblk")
                nc.sync.dma_start(wb[:], wqk[pi].rearrange("c ct p -> c (ct p)"))
                ps = psH.tile([128, LQ], FP32)
                for ci in range(16):
                    nc.tensor.matmul(ps[:], wb[:, ci * 128:(ci + 1) * 128],
                                     hT[:, ci], start=(ci == 0), stop=(ci == 15))
                nc.scalar.activation(qkT[:, pi], ps[:], AF.Identity,
                                     bias=sm[:, S_QKB + pi:S_QKB + pi + 1])
            if debug:
                qf = pA.tile([128, 32, LQ], FP32, tag="qf")
                for pi in range(32):
                    nc.vector.tensor_copy(qf[:, pi], qkT[:, pi])
                nc.sync.dma_start(dbg["qkT"].rearrange("ct p f -> p ct f"), qf[:])

            wv_t = pA.tile([128, 16, DM], BF16)
            nc.sync.dma_start(wv_t[:], wv.rearrange("ct p f -> p ct f"))
            v_t = pA.tile([128, 4, DM], BF16)
            for tj in range(4):
                ps = psV.tile([128, DM], FP32)
                for ci in range(16):
                    for fc in range(4):
                        nc.tensor.matmul(
                            ps[:, fc * 512:(fc + 1) * 512],
                            hT[:, ci, tj * 128:(tj + 1) * 128],
                            wv_t[:, ci, fc * 512:(fc + 1) * 512],
                            start=(ci == 0), stop=False)
                for fc in range(4):
                    nc.tensor.matmul(ps[:, fc * 512:(fc + 1) * 512],
                                     ones_row[:],
                                     vb_t[:, fc * 512:(fc + 1) * 512],
                                     start=False, stop=True)
                nc.scalar.copy(v_t[:, tj], ps[:])

            for j in range(8):
                nc.sync.dma_start(
                    a2a_qk_i[j, 0].rearrange("p (rt f) -> p rt f", rt=2),
                    qkT[:, 2 * j:2 * j + 2, :])
                nc.sync.dma_start(
                    a2a_qk_i[j, 1].rearrange("p (rt f) -> p rt f", rt=2),
                    qkT[:, 16 + 2 * j:16 + 2 * j + 2, :])
            nc.gpsimd.collective_compute(
                "AllToAll", ALU.bypass, replica_groups=[list(range(8))],
                ins=[a2a_qk_i.opt()], outs=[a2a_qk_o.opt()])
            for j in range(8):
                nc.sync.dma_start(
                    a2a_v_i[j].rearrange("p (tj f) -> p tj f", tj=4),
                    v_t[:, :, 256 * j:256 * (j + 1)])

        nc.gpsimd.collective_compute(
            "AllToAll", ALU.bypass, replica_groups=[list(range(8))],
            ins=[a2a_v_i.opt()], outs=[a2a_v_o.opt()])

        # ------------- Phase A2: attention (own global head, both batches) ----
        with tc.tile_pool(name="phAtt", bufs=1) as pAt, \
             tc.tile_pool(name="psS", bufs=2, space="PSUM") as psS, \
             tc.tile_pool(name="psC", bufs=1, space="PSUM") as psC, \
             tc.tile_pool(name="psR", bufs=1, space="PSUM") as psR:
            qT_o = pAt.tile([128, 2, 2, L], BF16)   # [b, hd-rowtile, L]
            kT_o = pAt.tile([128, 2, 2, L], BF16)
            v_o = pAt.tile([128, 2, 16, HD], BF16)  # [b, L-tile, hd]
            for b in range(2):
                for rt in range(2):
                    nc.sync.dma_start(
                        qT_o[:, b, rt].rearrange("p (i f) -> p i f", i=4),
                        a2a_qk_o[4 * b:4 * b + 4, 0, :,
                                 rt * LQ:(rt + 1) * LQ]
                        .rearrange("i p f -> p i f"))
                    nc.sync.dma_start(
                        kT_o[:, b, rt].rearrange("p (i f) -> p i f", i=4),
                        a2a_qk_o[4 * b:4 * b + 4, 1, :,
                                 rt * LQ:(rt + 1) * LQ]
                        .rearrange("i p f -> p i f"))
                for ii in range(4):
                    nc.sync.dma_start(
                        v_o[:, b, 4 * ii:4 * ii + 4, :],
                        a2a_v_o[4 * b + ii]
                        .rearrange("p (tj f) -> p tj f", tj=4))

            ctxT_o = pAt.tile([128, 2, 2, L], BF16)  # [b, hsub, q]
            pT = pAt.tile([128, 16, TH], BF16)
            for b in range(2):
                for qh in range(2):
                    for kt in range(16):
                        ps = psS.tile([128, TH], FP32)
                        for cc in range(2):
                            for fc in range(2):
                                nc.tensor.matmul(
                                    ps[:, fc * 512:(fc + 1) * 512],
                                    kT_o[:, b, cc, kt * 128:(kt + 1) * 128],
                                    qT_o[:, b, cc,
                                         qh * TH + fc * 512:
                                         qh * TH + (fc + 1) * 512],
                                    start=(cc == 0), stop=(cc == 1))
                        nc.scalar.activation(pT[:, kt], ps[:], AF.Exp,
                                             scale=0.0625)
                    psum_r = psR.tile([1, TH], FP32)
                    for kt in range(16):
                        for fc in range(2):
                            nc.tensor.matmul(
                                psum_r[:, fc * 512:(fc + 1) * 512],
                                ones_t[:], pT[:, kt, fc * 512:(fc + 1) * 512],
                                start=(kt == 0), stop=(kt == 15))
                    recip = pAt.tile([1, TH], FP32, tag="recip")
                    nc.vector.reciprocal(recip[:], psum_r[:])
                    rrep = pAt.tile([128, TH], FP32, tag="rrep")
                    nc.gpsimd.partition_broadcast(rrep[:], recip[:])
                    for hsub in range(2):
                        ps = psC.tile([128, TH], FP32)
                        for kt in range(16):
                            for fc in range(2):
                                nc.tensor.matmul(
                                    ps[:, fc * 512:(fc + 1) * 512],
                                    v_o[:, b, kt,
                                        hsub * 128:(hsub + 1) * 128],
                                    pT[:, kt, fc * 512:(fc + 1) * 512],
                                    start=(kt == 0), stop=(kt == 15))
                        nc.vector.tensor_tensor(
                            ctxT_o[:, b, hsub, qh * TH:(qh + 1) * TH],
                            ps[:], rrep[:], op=ALU.mult)
            for j in range(8):
                nc.sync.dma_start(
                    a2a_ctx_i[j].rearrange("p (rt f) -> p rt f", rt=2),
                    ctxT_o[:, j // 4, :, LQ * (j % 4):LQ * (j % 4 + 1)])

        nc.gpsimd.collective_compute(
            "AllToAll", ALU.bypass, replica_groups=[list(range(8))],
            ins=[a2a_ctx_i.opt()], outs=[a2a_ctx_o.opt()])

        # ---------------- Phase B: attn_out + LN + AG ----------------
        with tc.tile_pool(name="phB", bufs=1) as pB, \
             tc.tile_pool(name="wstreamB", bufs=3) as wsB, \
             tc.tile_pool(name="psB", bufs=4, space="PSUM") as psB, \
             tc.tile_pool(name="psStat", bufs=2, space="PSUM") as psStat:
            ctxA = pB.tile([128, 16, LQ], BF16)
            for j in range(8):
                nc.sync.dma_start(
                    ctxA[:, 2 * j:2 * j + 2, :],
                    a2a_ctx_o[j].rearrange("p (rt f) -> p rt f", rt=2))
            if debug:
                cf = pB.tile([128, 16, LQ], FP32, tag="cf")
                for pi in range(16):
                    nc.vector.tensor_copy(cf[:, pi], ctxA[:, pi])
                nc.sync.dma_start(dbg["ctxa"].rearrange("ct p f -> p ct f"),
                                  cf[:])

            aoT = pB.tile([128, 16, LQ], FP32)
            ao16 = pB.tile([128, 16, LQ], BF16)
            sq16 = pB.tile([128, 16, LQ], BF16)
            for pi in range(16):
                wb = wsB.tile([128, 16 * 128], BF16, tag="wblkB")
                nc.sync.dma_start(wb[:], wao[pi].rearrange("c ct p -> c (ct p)"))
                ps = psB.tile([128, LQ], FP32)
                for ci in range(16):
                    nc.tensor.matmul(ps[:], wb[:, ci * 128:(ci + 1) * 128],
                                     ctxA[:, ci], start=(ci == 0),
                                     stop=(ci == 15))
                nc.scalar.activation(aoT[:, pi], ps[:], AF.Identity,
                                     bias=sm[:, S_AOB + pi:S_AOB + pi + 1])
                nc.vector.tensor_copy(ao16[:, pi], aoT[:, pi])
                nc.scalar.activation(sq16[:, pi], ao16[:, pi], AF.Square)
            nc.sync.dma_start(ao_out.rearrange("ct p f -> p ct f"), aoT[:])
            if debug:
                nc.sync.dma_start(dbg["ao"].rearrange("ct p f -> p ct f"),
                                  aoT[:])
            ps_s = psStat.tile([1, LQ], FP32)
            ps_q = psStat.tile([1, LQ], FP32)
            for ci in range(16):
                nc.tensor.matmul(ps_s[:], ones_t[:], ao16[:, ci],
                                 start=(ci == 0), stop=(ci == 15))
            for ci in range(16):
                nc.tensor.matmul(ps_q[:], ones_t[:], sq16[:, ci],
                                 start=(ci == 0), stop=(ci == 15))
            mu = pB.tile([1, LQ], FP32, tag="mu")
            var = pB.tile([1, LQ], FP32, tag="var")
            nc.vector.tensor_scalar_mul(mu[:], ps_s[:], 1.0 / DM)
            nc.vector.tensor_scalar_mul(var[:], ps_q[:], 1.0 / DM)
            musq = pB.tile([1, LQ], FP32, tag="musq")
            nc.scalar.activation(musq[:], mu[:], AF.Square)
            nc.vector.tensor_sub(var[:], var[:], musq[:])
            sqv = pB.tile([1, LQ], FP32, tag="sqv")
            nc.scalar.activation(sqv[:], var[:], AF.Sqrt,
                                 bias=sm[0:1, S_EPS:S_EPS + 1])
            rstd = pB.tile([1, LQ], FP32, tag="rstd")
            nc.vector.reciprocal(rstd[:], sqv[:])
            mu_r = pB.tile([128, LQ], FP32, tag="mu_r")
            rs_r = pB.tile([128, LQ], FP32, tag="rs_r")
            nc.gpsimd.partition_broadcast(mu_r[:], mu[:])
            nc.gpsimd.partition_broadcast(rs_r[:], rstd[:])
            xnT = pB.tile([128, 16, LQ], BF16)
            xnf = pB.tile([128, LQ], FP32, tag="xnf")
            for pi in range(16):
                nc.vector.tensor_sub(xnf[:], aoT[:, pi], mu_r[:])
                nc.vector.tensor_tensor(xnT[:, pi], xnf[:], rs_r[:],
                                        op=ALU.mult)
            if debug:
                xf = pB.tile([128, 16, LQ], FP32, tag="xf")
                for pi in range(16):
                    nc.vector.tensor_copy(xf[:, pi], xnT[:, pi])
                nc.sync.dma_start(dbg["xn"].rearrange("ct p f -> p ct f"),
                                  xf[:])
            nc.sync.dma_start(
                ag_xn_i[0].rearrange("ct p f -> p ct f"), xnT[:, 0:8, :])
            nc.sync.dma_start(
                ag_xn_i[1].rearrange("ct p f -> p ct f"), xnT[:, 8:16, :])

        nc.gpsimd.collective_compute(
            "AllGather", ALU.bypass, replica_groups=GROUPS,
            ins=[ag_xn_i[0].opt()], outs=[ag_xn_o[0].opt()])
        nc.gpsimd.collective_compute(
            "AllGather", ALU.bypass, replica_groups=GROUPS,
            ins=[ag_xn_i[1].opt()], outs=[ag_xn_o[1].opt()])

        # ---------------- Phase C: in_proj, conv, x_proj ----------------
        with tc.tile_pool(name="psBig", bufs=2, space="PSUM") as psBig:
            with tc.tile_pool(name="phC", bufs=1) as pC, \
                 tc.tile_pool(name="wstreamC", bufs=3) as wsC:
                xnA = pC.tile([128, 16, L], BF16)
                for ci in range(16):
                    nc.sync.dma_start(
                        xnA[:, ci].rearrange("p (r f) -> p r f", r=4),
                        ag_xn_o[ci // 8, :, ci % 8].rearrange("r p f -> p r f"))
                u_t = pC.tile([128, 8, DC - 1 + L], BF16)
                for ct in range(8):
                    nc.gpsimd.memset(u_t[:, ct, 0:DC - 1], 0.0)
                for pi in range(16):
                    wb = wsC.tile([128, 16 * 128], BF16, tag="wblkC")
                    nc.sync.dma_start(wb[:],
                                      win[pi].rearrange("c ct p -> c (ct p)"))
                    ps = psBig.tile([128, L], FP32)
                    for ci in range(16):
                        for fc in range(4):
                            nc.tensor.matmul(
                                ps[:, fc * 512:(fc + 1) * 512],
                                wb[:, ci * 128:(ci + 1) * 128],
                                xnA[:, ci, fc * 512:(fc + 1) * 512],
                                start=(ci == 0), stop=(ci == 15))
                    if pi < 8:
                        nc.scalar.copy(u_t[:, pi, DC - 1:], ps[:])
                    else:
                        szt = pC.tile([128, L], BF16, tag="szt")
                        nc.scalar.activation(
                            szt[:], ps[:], AF.Silu,
                            bias=sm[:, S_BZ + pi - 8:S_BZ + pi - 7])
                        nc.sync.dma_start(sz_d[pi - 8], szt[:])
                    if debug:
                        uzf = pC.tile([128, L], FP32, tag="uzf")
                        nc.scalar.copy(uzf[:], ps[:])
                        nc.sync.dma_start(dbg["uz"][pi], uzf[:])

                uc_t = pC.tile([128, 8, L], BF16)
                cacc = pC.tile([128, L], FP32, tag="cacc")
                for ct in range(8):
                    nc.vector.tensor_scalar_mul(
                        cacc[:], u_t[:, ct, 0:L],
                        sm[:, S_CONVW + ct:S_CONVW + ct + 1])
                    for j in range(1, DC):
                        nc.vector.scalar_tensor_tensor(
                            cacc[:], u_t[:, ct, j:j + L],
                            sm[:, S_CONVW + 8 * j + ct:S_CONVW + 8 * j + ct + 1],
                            cacc[:], op0=ALU.mult, op1=ALU.add)
                    nc.scalar.activation(uc_t[:, ct], cacc[:], AF.Silu,
                                         bias=sm[:, S_CONVB + ct:S_CONVB + ct + 1])
                    nc.sync.dma_start(uc_d[ct], uc_t[:, ct])
                if debug:
                    ucf = pC.tile([128, 8, L], FP32, tag="ucf")
                    for ct in range(8):
                        nc.vector.tensor_copy(ucf[:, ct], uc_t[:, ct])
                    nc.sync.dma_start(dbg["uc"].rearrange("ct p f -> p ct f"),
                                      ucf[:])

                wxp_t = pC.tile([128, 8, 256], BF16, tag="wxp_t")
                nc.sync.dma_start(wxp_t[:], wxp.rearrange("ct p f -> p ct f"))
                xdp = pC.tile([128, 2, L], FP32, tag="xdp")
                for pi in range(2):
                    ps = psBig.tile([128, L], FP32)
                    for ci in range(8):
                        for fc in range(4):
                            nc.tensor.matmul(
                                ps[:, fc * 512:(fc + 1) * 512],
                                wxp_t[:, ci, pi * 128:(pi + 1) * 128],
                                uc_t[:, ci, fc * 512:(fc + 1) * 512],
                                start=(ci == 0), stop=(ci == 7))
                    nc.scalar.copy(xdp[:, pi], ps[:])
                nc.sync.dma_start(ar_xd_i.rearrange("r p f -> p r f"), xdp[:])

        nc.gpsimd.collective_compute(
            "AllReduce", ALU.add, replica_groups=GROUPS,
            ins=[ar_xd_i.opt()], outs=[ar_xd_o.opt()])

        # ---------------- Phase D: dt_proj + softplus + delta*uc ----------
        with tc.tile_pool(name="phD", bufs=1) as pD, \
             tc.tile_pool(name="psD", bufs=2, space="PSUM") as psD:
            xdt = pD.tile([128, 2, L], FP32)
            nc.sync.dma_start(xdt[:], ar_xd_o.rearrange("r p f -> p r f"))
            if debug:
                nc.sync.dma_start(dbg["xdbl"].rearrange("r p f -> p r f"),
                                  xdt[:])
            dt16 = pD.tile([128, L], BF16, tag="dt16")
            nc.vector.tensor_copy(dt16[:], xdt[:, 0])
            bc16 = pD.tile([32, L], BF16, tag="bc16")
            nc.vector.tensor_copy(bc16[:], xdt[0:32, 1])
            nc.sync.dma_start(bc_bf, bc16[:])
            wdt_t = pD.tile([128, 8, 128], BF16, tag="wdt_t")
            nc.sync.dma_start(wdt_t[:], wdt)
            for ct in range(8):
                ps = psD.tile([128, L], FP32)
                for fc in range(4):
                    nc.tensor.matmul(ps[:, fc * 512:(fc + 1) * 512],
                                     wdt_t[:, ct],
                                     dt16[:, fc * 512:(fc + 1) * 512],
                                     start=True, stop=True)
                e_f = pD.tile([128, L], FP32, tag="e_f")
                nc.scalar.activation(e_f[:], ps[:], AF.Exp,
                                     bias=sm[:, S_DTB + ct:S_DTB + ct + 1])
                del_f = pD.tile([128, L], FP32, tag="del_f")
                nc.scalar.activation(del_f[:], e_f[:], AF.Ln, bias=1.0)
                nc.sync.dma_start(delta_d[ct], del_f[:])
                uc_s = pD.tile([128, L], BF16, tag="uc_s")
                nc.sync.dma_start(uc_s[:], uc_d[ct])
                du_t = pD.tile([128, L], BF16, tag="du_t")
                nc.vector.tensor_tensor(du_t[:], del_f[:], uc_s[:],
                                        op=ALU.mult)
                nc.sync.dma_start(du_d[ct], du_t[:])
                if debug:
                    nc.sync.dma_start(dbg["delta"][ct], del_f[:])

        # ---------------- Phase E: selective scan ----------------
        # Full-sequence hardware scans per (ch-tile, state): [128, 2048].
        # dA on ACT, dBu on DVE, Ch mostly on GPSIMD, y-reduce on PE.
        with tc.tile_pool(name="phE", bufs=1) as pE, \
             tc.tile_pool(name="bcs", bufs=3) as pBc, \
             tc.tile_pool(name="scanst", bufs=3) as pScan, \
             tc.tile_pool(name="psY", bufs=1, space="PSUM") as psY:
            for ct in range(8):
                del_t = pScan.tile([128, L], FP32, tag="del_t")
                nc.sync.dma_start(del_t[:], delta_d[ct])
                du_tt = pScan.tile([128, L], BF16, tag="du_tt")
                nc.sync.dma_start(du_tt[:], du_d[ct])
                psy = psY.tile([128, L], FP32)
                for st in range(DS):
                    brep = pBc.tile([128, L], BF16, tag="brep")
                    nc.sync.dma_start(brep[:],
                                      bc_bf[st:st + 1, :].broadcast_to((128, L)))
                    crep = pBc.tile([128, L], BF16, tag="crep")
                    nc.sync.dma_start(
                        crep[:],
                        bc_bf[DS + st:DS + st + 1, :].broadcast_to((128, L)))
                    dA = pScan.tile([128, L], BF16, tag="dA")
                    nc.scalar.activation(dA[:], del_t[:], AF.Exp,
                                         scale=-float(st + 1))
                    dBu = pScan.tile([128, L], BF16, tag="dBu")
                    nc.vector.tensor_tensor(dBu[:], du_tt[:], brep[:],
                                            op=ALU.mult)
                    h_t = pScan.tile([128, L], BF16, tag="h_t")
                    nc.vector.tensor_tensor_scan(h_t[:], dA[:], dBu[:], 0.0,
                                                 op0=ALU.mult, op1=ALU.add)
                    ch = pScan.tile([128, L], BF16, tag="ch")
                    nc.vector.tensor_tensor(ch[:], h_t[:], crep[:],
                                            op=ALU.mult)
                    for fc in range(4):
                        nc.tensor.matmul(psy[:, fc * 512:(fc + 1) * 512],
                                         ident[:],
                                         ch[:, fc * 512:(fc + 1) * 512],
                                         start=(st == 0), stop=(st == DS - 1))
                uc_tt = pScan.tile([128, L], BF16, tag="uc_tt")
                nc.sync.dma_start(uc_tt[:], uc_d[ct])
                sz_tt = pScan.tile([128, L], BF16, tag="sz_tt")
                nc.sync.dma_start(sz_tt[:], sz_d[ct])
                yt = pScan.tile([128, L], BF16, tag="yt")
                nc.vector.scalar_tensor_tensor(
                    yt[:], uc_tt[:], sm[:, S_D + ct:S_D + ct + 1], psy[:],
                    op0=ALU.mult, op1=ALU.add)
                yg = pScan.tile([128, L], BF16, tag="yg")
                nc.vector.tensor_tensor(yg[:], yt[:], sz_tt[:], op=ALU.mult)
                nc.sync.dma_start(y_d[ct], yg[:])
                if debug:
                    ydf = pScan.tile([128, L], FP32, tag="ydf")
                    nc.scalar.copy(ydf[:], psy[:])
                    nc.sync.dma_start(dbg["y"][ct], ydf[:])

        # ---------------- Phase F: out_proj ----------------
        with tc.tile_pool(name="phF", bufs=1) as pF, \
             tc.tile_pool(name="otp", bufs=3) as pOt, \
             tc.tile_pool(name="wstreamF", bufs=4) as wsF, \
             tc.tile_pool(name="psF", bufs=4, space="PSUM") as psF:
            y_t = pF.tile([128, 8, L], BF16)
            for ct in range(8):
                nc.sync.dma_start(y_t[:, ct], y_d[ct])
            for pi in range(16):
                wb = wsF.tile([128, 8 * 128], BF16, tag="wblkF")
                nc.sync.dma_start(wb[:], wout[pi].rearrange("c ct p -> c (ct p)"))
                ot = pOt.tile([128, L], FP32, tag="ot")
                for fc in range(4):
                    ps = psF.tile([128, 512], FP32)
                    for ci in range(8):
                        nc.tensor.matmul(
                            ps[:], wb[:, ci * 128:(ci + 1) * 128],
                            y_t[:, ci, fc * 512:(fc + 1) * 512],
                            start=(ci == 0), stop=(ci == 7))
                    nc.scalar.copy(ot[:, fc * 512:(fc + 1) * 512], ps[:])
                nc.sync.dma_start(out_part[pi], ot[:])

    return nc, dbg


_CACHE = {}


def _prep(inputs):
    import ml_dtypes
    f32 = np.float32
    b16 = lambda a: np.ascontiguousarray(
        np.asarray(a, f32).astype(ml_dtypes.bfloat16))
    x = np.asarray(inputs["x"], f32)
    proj_w = np.asarray(inputs["proj_w"], f32)
    proj_b = np.asarray(inputs["proj_b"], f32)
    qkv_w = np.asarray(inputs["qkv_w"], f32)
    qkv_b = np.asarray(inputs["qkv_b"], f32)
    attn_out_w = np.asarray(inputs["attn_out_w"], f32)
    attn_out_b = np.asarray(inputs["attn_out_b"], f32)
    ln_g = np.asarray(inputs["ln_g"], f32)
    ln_b = np.asarray(inputs["ln_b"], f32)
    in_proj_w = np.asarray(inputs["in_proj_w"], f32)
    conv_w = np.asarray(inputs["conv_w"], f32)
    conv_b = np.asarray(inputs["conv_b"], f32)
    x_proj_w = np.asarray(inputs["x_proj_w"], f32)
    dt_proj_w = np.asarray(inputs["dt_proj_w"], f32)
    dt_proj_b = np.asarray(inputs["dt_proj_b"], f32)
    D_in = np.asarray(inputs["D"], f32)
    out_proj_w = np.asarray(inputs["out_proj_w"], f32)

    w_in_eff = in_proj_w * ln_g[None, :]
    bias_uz = in_proj_w @ ln_b

    shared = {
        "wproj": b16(_col_blocks(proj_w.T, 16, 16)),
        "wqk": b16(_col_blocks(qkv_w[:2 * DM].T, 16, 32)),
        "wv": b16(_row_tiles(qkv_w[2 * DM:3 * DM].T, 16)),
        "wao": b16(_col_blocks(attn_out_w.T, 16, 16)),
        "vbias": b16(qkv_b[2 * DM:].reshape(1, DM)),
        "identin": b16(np.eye(128, dtype=f32)),
    }
    in_maps = []
    for c in range(NCORES):
        b, q = c // 4, c % 4
        chs = slice(CH * q, CH * (q + 1))
        m = dict(shared)
        m["xT"] = b16(_row_tiles(x[b].T[:, LQ * q:LQ * (q + 1)], 16))
        w_in_own = np.concatenate(
            [w_in_eff[chs], w_in_eff[DI:][chs]], axis=0)
        m["win"] = b16(_col_blocks(w_in_own.T, 16, 16))
        wxp_p = np.zeros((CH, 256), f32)
        wxp_p[:, :DTR + 2 * DS] = x_proj_w[:, chs].T
        m["wxp"] = b16(wxp_p.reshape(8, 128, 256))
        m["wdt"] = b16(dt_proj_w[chs].T.reshape(DTR, 8, 128))
        m["wout"] = b16(_col_blocks(out_proj_w[:, chs].T, 8, 16))
        sml = np.zeros((128, NSMALL), f32)
        sml[:, S_PROJB:S_PROJB + 16] = proj_b.reshape(16, 128).T
        sml[:, S_QKB:S_QKB + 32] = qkv_b[:2 * DM].reshape(32, 128).T
        sml[:, S_AOB:S_AOB + 16] = attn_out_b.reshape(16, 128).T
        sml[:, S_BZ:S_BZ + 8] = bias_uz[DI:][chs].reshape(8, 128).T
        for j in range(DC):
            sml[:, S_CONVW + 8 * j:S_CONVW + 8 * (j + 1)] = \
                conv_w[chs, j].reshape(8, 128).T
        conv_b_eff = conv_b[chs] + bias_uz[:DI][chs] * conv_w[chs].sum(-1)
        sml[:, S_CONVB:S_CONVB + 8] = conv_b_eff.reshape(8, 128).T
        sml[:, S_DTB:S_DTB + 8] = dt_proj_b[chs].reshape(8, 128).T
        sml[:, S_D:S_D + 8] = D_in[chs].reshape(8, 128).T
        sml[:, S_EPS] = 1e-5
        m["smalls"] = sml
        in_maps.append(m)
    return in_maps


def run(inputs, debug=False, trace=False):
    key = ("dbg" if debug else "prog")
    if key not in _CACHE:
        prog = build_program(debug=debug)
        prog[0].compile()
        _CACHE[key] = prog
    nc, dbg = _CACHE[key]
    in_maps = _prep(inputs)
    res = bass_utils.run_bass_kernel_spmd(
        nc, in_maps, core_ids=list(range(NCORES)), trace=trace)
    out = np.zeros((BATCH, L, DM), np.float32)
    for b in range(BATCH):
        accT = None
        for c in GROUPS[b]:
            p = res.results[c]["out_part"].reshape(DM, DM)
            accT = p.copy() if accT is None else accT + p
        for c in GROUPS[b]:
            q = c % 4
            ao = res.results[c]["ao_out"].reshape(DM, LQ)
            accT[:, LQ * q:LQ * (q + 1)] += ao
        out[b] = accT.T
    return out, res


def kernel(**inputs):
    out, _ = run(inputs)
    return out


# revision 12
# speedup vs baseline: 1.0194x; 1.0194x over previous
"""Trainium2 Bass kernel for the attention+Mamba hybrid block.

Sharding over 8 NeuronCores: 2 batch groups x 4 cores (replica groups
[[0..3],[4..7]]). Within a group, proj/qkv are L-sharded, scores/softmax/ctx
head-sharded (AllToAll exchanges), LN L-sharded, the Mamba block
channel-sharded (d_inner/4 per core) with the selective scan run as hardware
tensor_tensor_scan over (channel x state) lanes.

Core c: batch b=c//4, quarter q=c%4
  - owns L rows   [512q, 512q+512)
  - owns heads    {2q, 2q+1}
  - owns channels [1024q, 1024q+1024)
Outputs per core: out_part = out^T partial [2048, 2048] fp32 over its channel
slice, ao_out = attn^T [2048, 512] fp32 for its L rows. Host: per batch,
out^T = sum(out_part) ; out^T[:, own cols] += ao_out ; transpose.
"""
import numpy as np
from contextlib import ExitStack

import concourse.bass as bass
import concourse.mybir as mybir
import concourse.tile as tile
import concourse.bacc as bacc
from concourse import bass_utils

AF = mybir.ActivationFunctionType
ALU = mybir.AluOpType
FP32 = mybir.dt.float32
BF16 = mybir.dt.bfloat16

DM = 2048
L = 2048
BATCH = 2
H = 8
HD = 256
DI = 4096
DS = 16
DC = 4
DTR = 128
NCORES = 8
GROUPS = [[0, 1, 2, 3], [4, 5, 6, 7]]
LQ = 512
CH = 1024
TH = 1024

S_PROJB = 0
S_QKB = 16
S_AOB = 48
S_BZ = 64
S_CONVW = 72
S_CONVB = 104
S_DTB = 112
S_D = 120
S_EPS = 128
NSMALL = 132


def _col_blocks(wT, n_ct, n_pt, pw=128):
    c, p = wT.shape
    assert c == n_ct * 128 and p == n_pt * pw
    return np.ascontiguousarray(
        wT.reshape(n_ct, 128, n_pt, pw).transpose(2, 1, 0, 3))


def _row_tiles(a, n_rt):
    r, f = a.shape
    assert r == n_rt * 128
    return np.ascontiguousarray(a.reshape(n_rt, 128, f))


def build_program(debug=False):
    nc = bacc.Bacc("TRN2", target_bir_lowering=False, debug=False,
                   num_devices=NCORES)

    def din(name, shape, dt=BF16):
        return nc.dram_tensor(name, shape, dt, kind="ExternalInput").ap()

    xT = din("xT", [16, 128, LQ])
    wproj = din("wproj", [16, 128, 16, 128])
    wqk = din("wqk", [32, 128, 16, 128])
    wv = din("wv", [16, 128, DM])
    wao = din("wao", [16, 128, 16, 128])
    win = din("win", [16, 128, 16, 128])
    wxp = din("wxp", [8, 128, 256])
    wdt = din("wdt", [128, 8, 128])
    wout = din("wout", [16, 128, 8, 128])
    vbias = din("vbias", [1, DM])
    identin = din("identin", [128, 128])
    smalls = din("smalls", [128, NSMALL], FP32)

    out_part = nc.dram_tensor("out_part", [16, 128, DM], FP32,
                              kind="ExternalOutput").ap()
    ao_out = nc.dram_tensor("ao_out", [16, 128, LQ], FP32,
                            kind="ExternalOutput").ap()

    def dint(name, shape, dt=BF16, shared=False):
        return nc.dram_tensor(name, shape, dt, kind="Internal",
                              addr_space="Shared" if shared else "Local").ap()

    a2a_qk_i = dint("a2a_qk_i", [8, 2, 128, 1024])
    a2a_qk_o = dint("a2a_qk_o", [8, 2, 128, 1024])
    a2a_v_i = dint("a2a_v_i", [8, 128, 1024])
    a2a_v_o = dint("a2a_v_o", [8, 128, 1024])
    a2a_ctx_i = dint("a2a_ctx_i", [8, 128, 1024])
    a2a_ctx_o = dint("a2a_ctx_o", [8, 128, 1024])
    ag_xn_i = dint("ag_xn_i", [2, 8, 128, LQ])
    ag_xn_o = dint("ag_xn_o", [2, 4, 8, 128, LQ])
    ar_xd_i = dint("ar_xd_i", [2, 128, L], FP32)
    ar_xd_o = dint("ar_xd_o", [2, 128, L], FP32)
    bc_bf = dint("bc_bf", [32, L])
    delta_d = dint("delta_d", [8, 128, L], FP32)
    du_d = dint("du_d", [8, 128, L])
    uc_d = dint("uc_d", [8, 128, L])
    sz_d = dint("sz_d", [8, 128, L])
    y_d = dint("y_d", [8, 128, L])

    dbg = {}
    if debug:
        for nm, shp in [("hT", [16, 128, LQ]), ("qkT", [32, 128, LQ]),
                        ("ao", [16, 128, LQ]), ("xn", [16, 128, LQ]),
                        ("uz", [16, 128, L]), ("uc", [8, 128, L]),
                        ("delta", [8, 128, L]), ("xdbl", [2, 128, L]),
                        ("y", [8, 128, L]), ("ctxa", [16, 128, LQ])]:
            dbg[nm] = nc.dram_tensor("dbg_" + nm, shp, FP32,
                                     kind="ExternalOutput").ap()

    with tile.TileContext(nc) as tc, ExitStack() as top:
        persist = top.enter_context(tc.tile_pool(name="persist", bufs=1))
        sm = persist.tile([128, NSMALL], FP32)
        nc.gpsimd.dma_start(sm[:], smalls)
        ones_t = persist.tile([128, 1], BF16)
        nc.gpsimd.memset(ones_t[:], 1.0)
        ones_row = persist.tile([1, 128], BF16)
        nc.gpsimd.memset(ones_row[:], 1.0)
        ident = persist.tile([128, 128], BF16)
        nc.gpsimd.dma_start(ident[:], identin)
        vb_t = persist.tile([1, DM], BF16)
        nc.gpsimd.dma_start(vb_t[:], vbias)

        # ---------------- Phase A: proj, qkv, v ----------------
        with tc.tile_pool(name="phA", bufs=1) as pA, \
             tc.tile_pool(name="wstream", bufs=3) as wsp, \
             tc.tile_pool(name="psH", bufs=4, space="PSUM") as psH, \
             tc.tile_pool(name="psV", bufs=1, space="PSUM") as psV:
            xT_t = pA.tile([128, 16, LQ], BF16)
            nc.sync.dma_start(xT_t[:], xT.rearrange("ct p f -> p ct f"))
            hT = pA.tile([128, 16, LQ], BF16)
            for pi in range(16):
                wb = wsp.tile([128, 16 * 128], BF16, tag="wblk")
                nc.sync.dma_start(wb[:], wproj[pi].rearrange("c ct p -> c (ct p)"))
                ps = psH.tile([128, LQ], FP32)
                for ci in range(16):
                    nc.tensor.matmul(ps[:], wb[:, ci * 128:(ci + 1) * 128],
                                     xT_t[:, ci], start=(ci == 0), stop=(ci == 15))
                nc.scalar.activation(hT[:, pi], ps[:], AF.Identity,
                                     bias=sm[:, S_PROJB + pi:S_PROJB + pi + 1])
            if debug:
                hTf = pA.tile([128, 16, LQ], FP32, tag="hTf")
                for pi in range(16):
                    nc.vector.tensor_copy(hTf[:, pi], hT[:, pi])
                nc.sync.dma_start(dbg["hT"].rearrange("ct p f -> p ct f"), hTf[:])

            qkT = pA.tile([128, 32, LQ], BF16)
            for pi in range(32):
                wb = wsp.tile([128, 16 * 128], BF16, tag="wblk")
                nc.sync.dma_start(wb[:], wqk[pi].rearrange("c ct p -> c (ct p)"))
                ps = psH.tile([128, LQ], FP32)
                for ci in range(16):
                    nc.tensor.matmul(ps[:], wb[:, ci * 128:(ci + 1) * 128],
                                     hT[:, ci], start=(ci == 0), stop=(ci == 15))
                nc.scalar.activation(qkT[:, pi], ps[:], AF.Identity,
                                     bias=sm[:, S_QKB + pi:S_QKB + pi + 1])
            if debug:
                qf = pA.tile([128, 32, LQ], FP32, tag="qf")
                for pi in range(32):
                    nc.vector.tensor_copy(qf[:, pi], qkT[:, pi])
                nc.sync.dma_start(dbg["qkT"].rearrange("ct p f -> p ct f"), qf[:])

            wv_t = pA.tile([128, 16, DM], BF16)
            nc.sync.dma_start(wv_t[:], wv.rearrange("ct p f -> p ct f"))
            v_t = pA.tile([128, 4, DM], BF16)
            for tj in range(4):
                ps = psV.tile([128, DM], FP32)
                for ci in range(16):
                    for fc in range(4):
                        nc.tensor.matmul(
                            ps[:, fc * 512:(fc + 1) * 512],
                            hT[:, ci, tj * 128:(tj + 1) * 128],
                            wv_t[:, ci, fc * 512:(fc + 1) * 512],
                            start=(ci == 0), stop=False)
                for fc in range(4):
                    nc.tensor.matmul(ps[:, fc * 512:(fc + 1) * 512],
                                     ones_row[:],
                                     vb_t[:, fc * 512:(fc + 1) * 512],
                                     start=False, stop=True)
                nc.scalar.copy(v_t[:, tj], ps[:])

            for j in range(8):
                nc.sync.dma_start(
                    a2a_qk_i[j, 0].rearrange("p (rt f) -> p rt f", rt=2),
                    qkT[:, 2 * j:2 * j + 2, :])
                nc.sync.dma_start(
                    a2a_qk_i[j, 1].rearrange("p (rt f) -> p rt f", rt=2),
                    qkT[:, 16 + 2 * j:16 + 2 * j + 2, :])
            nc.gpsimd.collective_compute(
                "AllToAll", ALU.bypass, replica_groups=[list(range(8))],
                ins=[a2a_qk_i.opt()], outs=[a2a_qk_o.opt()])
            for j in range(8):
                nc.sync.dma_start(
                    a2a_v_i[j].rearrange("p (tj f) -> p tj f", tj=4),
                    v_t[:, :, 256 * j:256 * (j + 1)])

        nc.gpsimd.collective_compute(
            "AllToAll", ALU.bypass, replica_groups=[list(range(8))],
            ins=[a2a_v_i.opt()], outs=[a2a_v_o.opt()])

        # ------------- Phase A2: attention (own global head, both batches) ----
        with tc.tile_pool(name="phAtt", bufs=1) as pAt, \
             tc.tile_pool(name="psS", bufs=2, space="PSUM") as psS, \
             tc.tile_pool(name="psC", bufs=1, space="PSUM") as psC, \
             tc.tile_pool(name="psR", bufs=1, space="PSUM") as psR:
            qT_o = pAt.tile([128, 2, 2, L], BF16)   # [b, hd-rowtile, L]
            kT_o = pAt.tile([128, 2, 2, L], BF16)
            v_o = pAt.tile([128, 2, 16, HD], BF16)  # [b, L-tile, hd]
            for b in range(2):
                for rt in range(2):
                    nc.sync.dma_start(
                        qT_o[:, b, rt].rearrange("p (i f) -> p i f", i=4),
                        a2a_qk_o[4 * b:4 * b + 4, 0, :,
                                 rt * LQ:(rt + 1) * LQ]
                        .rearrange("i p f -> p i f"))
                    nc.sync.dma_start(
                        kT_o[:, b, rt].rearrange("p (i f) -> p i f", i=4),
                        a2a_qk_o[4 * b:4 * b + 4, 1, :,
                                 rt * LQ:(rt + 1) * LQ]
                        .rearrange("i p f -> p i f"))
                for ii in range(4):
                    nc.sync.dma_start(
                        v_o[:, b, 4 * ii:4 * ii + 4, :],
                        a2a_v_o[4 * b + ii]
                        .rearrange("p (tj f) -> p tj f", tj=4))

            ctxT_o = pAt.tile([128, 2, 2, L], BF16)  # [b, hsub, q]
            pT = pAt.tile([128, 16, TH], BF16)
            for b in range(2):
                for qh in range(2):
                    for kt in range(16):
                        ps = psS.tile([128, TH], FP32)
                        for cc in range(2):
                            for fc in range(2):
                                nc.tensor.matmul(
                                    ps[:, fc * 512:(fc + 1) * 512],
                                    kT_o[:, b, cc, kt * 128:(kt + 1) * 128],
                                    qT_o[:, b, cc,
                                         qh * TH + fc * 512:
                                         qh * TH + (fc + 1) * 512],
                                    start=(cc == 0), stop=(cc == 1))
                        nc.scalar.activation(pT[:, kt], ps[:], AF.Exp,
                                             scale=0.0625)
                    psum_r = psR.tile([1, TH], FP32)
                    for kt in range(16):
                        for fc in range(2):
                            nc.tensor.matmul(
                                psum_r[:, fc * 512:(fc + 1) * 512],
                                ones_t[:], pT[:, kt, fc * 512:(fc + 1) * 512],
                                start=(kt == 0), stop=(kt == 15))
                    recip = pAt.tile([1, TH], FP32, tag="recip")
                    nc.vector.reciprocal(recip[:], psum_r[:])
                    rrep = pAt.tile([128, TH], FP32, tag="rrep")
                    nc.gpsimd.partition_broadcast(rrep[:], recip[:])
                    for hsub in range(2):
                        ps = psC.tile([128, TH], FP32)
                        for kt in range(16):
                            for fc in range(2):
                                nc.tensor.matmul(
                                    ps[:, fc * 512:(fc + 1) * 512],
                                    v_o[:, b, kt,
                                        hsub * 128:(hsub + 1) * 128],
                                    pT[:, kt, fc * 512:(fc + 1) * 512],
                                    start=(kt == 0), stop=(kt == 15))
                        nc.vector.tensor_tensor(
                            ctxT_o[:, b, hsub, qh * TH:(qh + 1) * TH],
                            ps[:], rrep[:], op=ALU.mult)
            for j in range(8):
                nc.sync.dma_start(
                    a2a_ctx_i[j].rearrange("p (rt f) -> p rt f", rt=2),
                    ctxT_o[:, j // 4, :, LQ * (j % 4):LQ * (j % 4 + 1)])

        nc.gpsimd.collective_compute(
            "AllToAll", ALU.bypass, replica_groups=[list(range(8))],
            ins=[a2a_ctx_i.opt()], outs=[a2a_ctx_o.opt()])

        # ---------------- Phase B: attn_out + LN + AG ----------------
        with tc.tile_pool(name="phB", bufs=1) as pB, \
             tc.tile_pool(name="wstreamB", bufs=3) as wsB, \
             tc.tile_pool(name="psB", bufs=4, space="PSUM") as psB, \
             tc.tile_pool(name="psStat", bufs=2, space="PSUM") as psStat:
            ctxA = pB.tile([128, 16, LQ], BF16)
            for j in range(8):
                nc.sync.dma_start(
                    ctxA[:, 2 * j:2 * j + 2, :],
                    a2a_ctx_o[j].rearrange("p (rt f) -> p rt f", rt=2))
            if debug:
                cf = pB.tile([128, 16, LQ], FP32, tag="cf")
                for pi in range(16):
                    nc.vector.tensor_copy(cf[:, pi], ctxA[:, pi])
                nc.sync.dma_start(dbg["ctxa"].rearrange("ct p f -> p ct f"),
                                  cf[:])

            aoT = pB.tile([128, 16, LQ], FP32)
            ao16 = pB.tile([128, 16, LQ], BF16)
            sq16 = pB.tile([128, 16, LQ], BF16)
            for pi in range(16):
                wb = wsB.tile([128, 16 * 128], BF16, tag="wblkB")
                nc.sync.dma_start(wb[:], wao[pi].rearrange("c ct p -> c (ct p)"))
                ps = psB.tile([128, LQ], FP32)
                for ci in range(16):
                    nc.tensor.matmul(ps[:], wb[:, ci * 128:(ci + 1) * 128],
                                     ctxA[:, ci], start=(ci == 0),
                                     stop=(ci == 15))
                nc.scalar.activation(aoT[:, pi], ps[:], AF.Identity,
                                     bias=sm[:, S_AOB + pi:S_AOB + pi + 1])
                nc.vector.tensor_copy(ao16[:, pi], aoT[:, pi])
                nc.scalar.activation(sq16[:, pi], ao16[:, pi], AF.Square)
            nc.sync.dma_start(ao_out.rearrange("ct p f -> p ct f"), aoT[:])
            if debug:
                nc.sync.dma_start(dbg["ao"].rearrange("ct p f -> p ct f"),
                                  aoT[:])
            ps_s = psStat.tile([1, LQ], FP32)
            ps_q = psStat.tile([1, LQ], FP32)
            for ci in range(16):
                nc.tensor.matmul(ps_s[:], ones_t[:], ao16[:, ci],
                                 start=(ci == 0), stop=(ci == 15))
            for ci in range(16):
                nc.tensor.matmul(ps_q[:], ones_t[:], sq16[:, ci],
                                 start=(ci == 0), stop=(ci == 15))
            mu = pB.tile([1, LQ], FP32, tag="mu")
            var = pB.tile([1, LQ], FP32, tag="var")
            nc.vector.tensor_scalar_mul(mu[:], ps_s[:], 1.0 / DM)
            nc.vector.tensor_scalar_mul(var[:], ps_q[:], 1.0 / DM)
            musq = pB.tile([1, LQ], FP32, tag="musq")
            nc.scalar.activation(musq[:], mu[:], AF.Square)
            nc.vector.tensor_sub(var[:], var[:], musq[:])
            sqv = pB.tile([1, LQ], FP32, tag="sqv")
            nc.scalar.activation(sqv[:], var[:], AF.Sqrt,
                                 bias=sm[0:1, S_EPS:S_EPS + 1])
            rstd = pB.tile([1, LQ], FP32, tag="rstd")
            nc.vector.reciprocal(rstd[:], sqv[:])
            mu_r = pB.tile([128, LQ], FP32, tag="mu_r")
            rs_r = pB.tile([128, LQ], FP32, tag="rs_r")
            nc.gpsimd.partition_broadcast(mu_r[:], mu[:])
            nc.gpsimd.partition_broadcast(rs_r[:], rstd[:])
            xnT = pB.tile([128, 16, LQ], BF16)
            xnf = pB.tile([128, LQ], FP32, tag="xnf")
            for pi in range(16):
                nc.vector.tensor_sub(xnf[:], aoT[:, pi], mu_r[:])
                nc.vector.tensor_tensor(xnT[:, pi], xnf[:], rs_r[:],
                                        op=ALU.mult)
            if debug:
                xf = pB.tile([128, 16, LQ], FP32, tag="xf")
                for pi in range(16):
                    nc.vector.tensor_copy(xf[:, pi], xnT[:, pi])
                nc.sync.dma_start(dbg["xn"].rearrange("ct p f -> p ct f"),
                                  xf[:])
            nc.sync.dma_start(
                ag_xn_i[0].rearrange("ct p f -> p ct f"), xnT[:, 0:8, :])
            nc.sync.dma_start(
                ag_xn_i[1].rearrange("ct p f -> p ct f"), xnT[:, 8:16, :])

        nc.gpsimd.collective_compute(
            "AllGather", ALU.bypass, replica_groups=GROUPS,
            ins=[ag_xn_i[0].opt()], outs=[ag_xn_o[0].opt()])
        nc.gpsimd.collective_compute(
            "AllGather", ALU.bypass, replica_groups=GROUPS,
            ins=[ag_xn_i[1].opt()], outs=[ag_xn_o[1].opt()])

        # ---------------- Phase C: in_proj, conv, x_proj ----------------
        with tc.tile_pool(name="psBig", bufs=2, space="PSUM") as psBig:
            with tc.tile_pool(name="phC", bufs=1) as pC, \
                 tc.tile_pool(name="wstreamC", bufs=3) as wsC:
                xnA = pC.tile([128, 16, L], BF16)
                for ci in range(16):
                    nc.sync.dma_start(
                        xnA[:, ci].rearrange("p (r f) -> p r f", r=4),
                        ag_xn_o[ci // 8, :, ci % 8].rearrange("r p f -> p r f"))
                u_t = pC.tile([128, 8, DC - 1 + L], BF16)
                for ct in range(8):
                    nc.gpsimd.memset(u_t[:, ct, 0:DC - 1], 0.0)
                for pi in range(16):
                    wb = wsC.tile([128, 16 * 128], BF16, tag="wblkC")
                    nc.sync.dma_start(wb[:],
                                      win[pi].rearrange("c ct p -> c (ct p)"))
                    ps = psBig.tile([128, L], FP32)
                    for ci in range(16):
                        for fc in range(4):
                            nc.tensor.matmul(
                                ps[:, fc * 512:(fc + 1) * 512],
                                wb[:, ci * 128:(ci + 1) * 128],
                                xnA[:, ci, fc * 512:(fc + 1) * 512],
                                start=(ci == 0), stop=(ci == 15))
                    if pi < 8:
                        nc.scalar.copy(u_t[:, pi, DC - 1:], ps[:])
                    else:
                        szt = pC.tile([128, L], BF16, tag="szt")
                        nc.scalar.activation(
                            szt[:], ps[:], AF.Silu,
                            bias=sm[:, S_BZ + pi - 8:S_BZ + pi - 7])
                        nc.sync.dma_start(sz_d[pi - 8], szt[:])
                    if debug:
                        uzf = pC.tile([128, L], FP32, tag="uzf")
                        nc.scalar.copy(uzf[:], ps[:])
                        nc.sync.dma_start(dbg["uz"][pi], uzf[:])

                uc_t = pC.tile([128, 8, L], BF16)
                cacc = pC.tile([128, L], FP32, tag="cacc")
                for ct in range(8):
                    nc.vector.tensor_scalar_mul(
                        cacc[:], u_t[:, ct, 0:L],
                        sm[:, S_CONVW + ct:S_CONVW + ct + 1])
                    for j in range(1, DC):
                        nc.vector.scalar_tensor_tensor(
                            cacc[:], u_t[:, ct, j:j + L],
                            sm[:, S_CONVW + 8 * j + ct:S_CONVW + 8 * j + ct + 1],
                            cacc[:], op0=ALU.mult, op1=ALU.add)
                    nc.scalar.activation(uc_t[:, ct], cacc[:], AF.Silu,
                                         bias=sm[:, S_CONVB + ct:S_CONVB + ct + 1])
                    nc.sync.dma_start(uc_d[ct], uc_t[:, ct])
                if debug:
                    ucf = pC.tile([128, 8, L], FP32, tag="ucf")
                    for ct in range(8):
                        nc.vector.tensor_copy(ucf[:, ct], uc_t[:, ct])
                    nc.sync.dma_start(dbg["uc"].rearrange("ct p f -> p ct f"),
                                      ucf[:])

                wxp_t = pC.tile([128, 8, 256], BF16, tag="wxp_t")
                nc.sync.dma_start(wxp_t[:], wxp.rearrange("ct p f -> p ct f"))
                xdp = pC.tile([128, 2, L], FP32, tag="xdp")
                for pi in range(2):
                    ps = psBig.tile([128, L], FP32)
                    for ci in range(8):
                        for fc in range(4):
                            nc.tensor.matmul(
                                ps[:, fc * 512:(fc + 1) * 512],
                                wxp_t[:, ci, pi * 128:(pi + 1) * 128],
                                uc_t[:, ci, fc * 512:(fc + 1) * 512],
                                start=(ci == 0), stop=(ci == 7))
                    nc.scalar.copy(xdp[:, pi], ps[:])
                nc.sync.dma_start(ar_xd_i.rearrange("r p f -> p r f"), xdp[:])

        nc.gpsimd.collective_compute(
            "AllReduce", ALU.add, replica_groups=GROUPS,
            ins=[ar_xd_i.opt()], outs=[ar_xd_o.opt()])

        # ---------------- Phase D: dt_proj + softplus + delta*uc ----------
        with tc.tile_pool(name="phD", bufs=1) as pD, \
             tc.tile_pool(name="psD", bufs=2, space="PSUM") as psD:
            xdt = pD.tile([128, 2, L], FP32)
            nc.sync.dma_start(xdt[:], ar_xd_o.rearrange("r p f -> p r f"))
            if debug:
                nc.sync.dma_start(dbg["xdbl"].rearrange("r p f -> p r f"),
                                  xdt[:])
            dt16 = pD.tile([128, L], BF16, tag="dt16")
            nc.vector.tensor_copy(dt16[:], xdt[:, 0])
            bc16 = pD.tile([32, L], BF16, tag="bc16")
            nc.vector.tensor_copy(bc16[:], xdt[0:32, 1])
            nc.sync.dma_start(bc_bf, bc16[:])
            wdt_t = pD.tile([128, 8, 128], BF16, tag="wdt_t")
            nc.sync.dma_start(wdt_t[:], wdt)
            for ct in range(8):
                ps = psD.tile([128, L], FP32)
                for fc in range(4):
                    nc.tensor.matmul(ps[:, fc * 512:(fc + 1) * 512],
                                     wdt_t[:, ct],
                                     dt16[:, fc * 512:(fc + 1) * 512],
                                     start=True, stop=True)
                e_f = pD.tile([128, L], FP32, tag="e_f")
                nc.scalar.activation(e_f[:], ps[:], AF.Exp,
                                     bias=sm[:, S_DTB + ct:S_DTB + ct + 1])
                del_f = pD.tile([128, L], FP32, tag="del_f")
                nc.scalar.activation(del_f[:], e_f[:], AF.Ln, bias=1.0)
                nc.sync.dma_start(delta_d[ct], del_f[:])
                uc_s = pD.tile([128, L], BF16, tag="uc_s")
                nc.sync.dma_start(uc_s[:], uc_d[ct])
                du_t = pD.tile([128, L], BF16, tag="du_t")
                nc.vector.tensor_tensor(du_t[:], del_f[:], uc_s[:],
                                        op=ALU.mult)
                nc.sync.dma_start(du_d[ct], du_t[:])
                if debug:
                    nc.sync.dma_start(dbg["delta"][ct], del_f[:])

        # ---------------- Phase E: selective scan ----------------
        # Full-sequence hardware scans per (ch-tile, state): [128, 2048].
        # dA on ACT, dBu on DVE, Ch mostly on GPSIMD, y-reduce on PE.
        with tc.tile_pool(name="phE", bufs=1) as pE, \
             tc.tile_pool(name="bcs", bufs=3) as pBc, \
             tc.tile_pool(name="scanst", bufs=3) as pScan, \
             tc.tile_pool(name="psY", bufs=1, space="PSUM") as psY:
            for ct in range(8):
                del_t = pScan.tile([128, L], FP32, tag="del_t")
                nc.sync.dma_start(del_t[:], delta_d[ct])
                du_tt = pScan.tile([128, L], BF16, tag="du_tt")
                nc.sync.dma_start(du_tt[:], du_d[ct])
                psy = psY.tile([128, L], FP32)
                for st in range(DS):
                    brep = pBc.tile([128, L], BF16, tag="brep")
                    nc.sync.dma_start(brep[:],
                                      bc_bf[st:st + 1, :].broadcast_to((128, L)))
                    crep = pBc.tile([128, L], BF16, tag="crep")
                    nc.sync.dma_start(
                        crep[:],
                        bc_bf[DS + st:DS + st + 1, :].broadcast_to((128, L)))
                    dA = pScan.tile([128, L], BF16, tag="dA")
                    nc.scalar.activation(dA[:], del_t[:], AF.Exp,
                                         scale=-float(st + 1))
                    dBu = pScan.tile([128, L], BF16, tag="dBu")
                    nc.vector.tensor_tensor(dBu[:], du_tt[:], brep[:],
                                            op=ALU.mult)
                    h_t = pScan.tile([128, L], BF16, tag="h_t")
                    nc.vector.tensor_tensor_scan(h_t[:], dA[:], dBu[:], 0.0,
                                                 op0=ALU.mult, op1=ALU.add)
                    ch = pScan.tile([128, L], BF16, tag="ch")
                    nc.vector.tensor_tensor(ch[:], h_t[:], crep[:],
                                            op=ALU.mult)
                    for fc in range(4):
                        nc.tensor.matmul(psy[:, fc * 512:(fc + 1) * 512],
                                         ident[:],
                                         ch[:, fc * 512:(fc + 1) * 512],
                                         start=(st == 0), stop=(st == DS - 1))
                uc_tt = pScan.tile([128, L], BF16, tag="uc_tt")
                nc.sync.dma_start(uc_tt[:], uc_d[ct])
                sz_tt = pScan.tile([128, L], BF16, tag="sz_tt")
                nc.sync.dma_start(sz_tt[:], sz_d[ct])
                yt = pScan.tile([128, L], BF16, tag="yt")
                nc.vector.scalar_tensor_tensor(
                    yt[:], uc_tt[:], sm[:, S_D + ct:S_D + ct + 1], psy[:],
                    op0=ALU.mult, op1=ALU.add)
                yg = pScan.tile([128, L], BF16, tag="yg")
                nc.vector.tensor_tensor(yg[:], yt[:], sz_tt[:], op=ALU.mult)
                nc.sync.dma_start(y_d[ct], yg[:])
                if debug:
                    ydf = pScan.tile([128, L], FP32, tag="ydf")
                    nc.scalar.copy(ydf[:], psy[:])
                    nc.sync.dma_start(dbg["y"][ct], ydf[:])

        # ---------------- Phase F: out_proj ----------------
        with tc.tile_pool(name="phF", bufs=1) as pF, \
             tc.tile_pool(name="otp", bufs=3) as pOt, \
             tc.tile_pool(name="wstreamF", bufs=4) as wsF, \
             tc.tile_pool(name="psF", bufs=4, space="PSUM") as psF:
            y_ts = []
            for ct in range(8):
                yti = pF.tile([128, L], BF16, tag=f"y{ct}")
                nc.sync.dma_start(yti[:], y_d[ct])
                y_ts.append(yti)
            for pi in range(16):
                wb = wsF.tile([128, 8 * 128], BF16, tag="wblkF")
                nc.sync.dma_start(wb[:], wout[pi].rearrange("c ct p -> c (ct p)"))
                ot = pOt.tile([128, L], FP32, tag="ot")
                for fc in range(4):
                    ps = psF.tile([128, 512], FP32)
                    for ci in range(8):
                        nc.tensor.matmul(
                            ps[:], wb[:, ci * 128:(ci + 1) * 128],
                            y_ts[ci][:, fc * 512:(fc + 1) * 512],
                            start=(ci == 0), stop=(ci == 7))
                    nc.scalar.copy(ot[:, fc * 512:(fc + 1) * 512], ps[:])
                nc.sync.dma_start(out_part[pi], ot[:])

    return nc, dbg


_CACHE = {}


def _prep(inputs):
    import ml_dtypes
    f32 = np.float32
    b16 = lambda a: np.ascontiguousarray(
        np.asarray(a, f32).astype(ml_dtypes.bfloat16))
    x = np.asarray(inputs["x"], f32)
    proj_w = np.asarray(inputs["proj_w"], f32)
    proj_b = np.asarray(inputs["proj_b"], f32)
    qkv_w = np.asarray(inputs["qkv_w"], f32)
    qkv_b = np.asarray(inputs["qkv_b"], f32)
    attn_out_w = np.asarray(inputs["attn_out_w"], f32)
    attn_out_b = np.asarray(inputs["attn_out_b"], f32)
    ln_g = np.asarray(inputs["ln_g"], f32)
    ln_b = np.asarray(inputs["ln_b"], f32)
    in_proj_w = np.asarray(inputs["in_proj_w"], f32)
    conv_w = np.asarray(inputs["conv_w"], f32)
    conv_b = np.asarray(inputs["conv_b"], f32)
    x_proj_w = np.asarray(inputs["x_proj_w"], f32)
    dt_proj_w = np.asarray(inputs["dt_proj_w"], f32)
    dt_proj_b = np.asarray(inputs["dt_proj_b"], f32)
    D_in = np.asarray(inputs["D"], f32)
    out_proj_w = np.asarray(inputs["out_proj_w"], f32)

    w_in_eff = in_proj_w * ln_g[None, :]
    bias_uz = in_proj_w @ ln_b

    shared = {
        "wproj": b16(_col_blocks(proj_w.T, 16, 16)),
        "wqk": b16(_col_blocks(qkv_w[:2 * DM].T, 16, 32)),
        "wv": b16(_row_tiles(qkv_w[2 * DM:3 * DM].T, 16)),
        "wao": b16(_col_blocks(attn_out_w.T, 16, 16)),
        "vbias": b16(qkv_b[2 * DM:].reshape(1, DM)),
        "identin": b16(np.eye(128, dtype=f32)),
    }
    in_maps = []
    for c in range(NCORES):
        b, q = c // 4, c % 4
        chs = slice(CH * q, CH * (q + 1))
        m = dict(shared)
        m["xT"] = b16(_row_tiles(x[b].T[:, LQ * q:LQ * (q + 1)], 16))
        w_in_own = np.concatenate(
            [w_in_eff[chs], w_in_eff[DI:][chs]], axis=0)
        m["win"] = b16(_col_blocks(w_in_own.T, 16, 16))
        wxp_p = np.zeros((CH, 256), f32)
        wxp_p[:, :DTR + 2 * DS] = x_proj_w[:, chs].T
        m["wxp"] = b16(wxp_p.reshape(8, 128, 256))
        m["wdt"] = b16(dt_proj_w[chs].T.reshape(DTR, 8, 128))
        m["wout"] = b16(_col_blocks(out_proj_w[:, chs].T, 8, 16))
        sml = np.zeros((128, NSMALL), f32)
        sml[:, S_PROJB:S_PROJB + 16] = proj_b.reshape(16, 128).T
        sml[:, S_QKB:S_QKB + 32] = qkv_b[:2 * DM].reshape(32, 128).T
        sml[:, S_AOB:S_AOB + 16] = attn_out_b.reshape(16, 128).T
        sml[:, S_BZ:S_BZ + 8] = bias_uz[DI:][chs].reshape(8, 128).T
        for j in range(DC):
            sml[:, S_CONVW + 8 * j:S_CONVW + 8 * (j + 1)] = \
                conv_w[chs, j].reshape(8, 128).T
        conv_b_eff = conv_b[chs] + bias_uz[:DI][chs] * conv_w[chs].sum(-1)
        sml[:, S_CONVB:S_CONVB + 8] = conv_b_eff.reshape(8, 128).T
        sml[:, S_DTB:S_DTB + 8] = dt_proj_b[chs].reshape(8, 128).T
        sml[:, S_D:S_D + 8] = D_in[chs].reshape(8, 128).T
        sml[:, S_EPS] = 1e-5
        m["smalls"] = sml
        in_maps.append(m)
    return in_maps


def run(inputs, debug=False, trace=False):
    key = ("dbg" if debug else "prog")
    if key not in _CACHE:
        prog = build_program(debug=debug)
        prog[0].compile()
        _CACHE[key] = prog
    nc, dbg = _CACHE[key]
    in_maps = _prep(inputs)
    res = bass_utils.run_bass_kernel_spmd(
        nc, in_maps, core_ids=list(range(NCORES)), trace=trace)
    out = np.zeros((BATCH, L, DM), np.float32)
    for b in range(BATCH):
        accT = None
        for c in GROUPS[b]:
            p = res.results[c]["out_part"].reshape(DM, DM)
            accT = p.copy() if accT is None else accT + p
        for c in GROUPS[b]:
            q = c % 4
            ao = res.results[c]["ao_out"].reshape(DM, LQ)
            accT[:, LQ * q:LQ * (q + 1)] += ao
        out[b] = accT.T
    return out, res


def kernel(**inputs):
    out, _ = run(inputs)
    return out
